# revision 21
# baseline (speedup 1.0000x reference)
"""Trainium2 Bass kernel: dual-softmax cross-attention bilinear forms.

Math (per batch b, a = corr[b] in [N, N], N = 3072):
    attn = exp(2a) * (1/rowsum_a) outer (1/colsum_a)
    fund1 = v1^T attn v1,  fund2^T = v2^T attn^T v2
Device computes, per core (4 batches x 2 row-halves = 8 cores), for its
half slab [NH=1536, N]:
    E1 = exp(a + B') fp16 on the scalar engine. B' is computed per core
    on the host (3.03 - amax) so that E1^2 <= 448 fits fp8 exactly; a
    arrives as fp8e4m3 (the quantization washes out through the dual
    normalization). rowsum via the activation accumulator; colsum
    partials via ones^T @ E1 matmuls on the PE's 4 column groups.
    e2 = fp8(E1^2): a plain DVE tensor_tensor square. The row
    normalization rides on the v side instead (vt = vK * rinv_row, a
    GPSIMD tensor_scalar per tile; host pre-scales v by K=4096 in fp16
    and divides X by K * e^{B'} afterwards), which is what lets the
    square be a two-source TT: for the first tile of each ipair the TT
    writes fp16 at DVE 2x_1P (~1.7us) and a GPSIMD-initiated casting
    DMA converts to fp8 (bit-exact RNE, verified); the second tile of
    each pair TTs straight to fp8 (1x) so the cast latency never gates
    the pair. DVE square load drops 41 -> ~33us, freeing it for
    mid-stream psum exports.
    X = e2^T @ vt -- fp8 DoubleRow matmul trios per (m, ipair):
    [0:256 | 256:512 | pos 16-wide window in a shared bank], ~278ns
    warm. PSUM (8 banks): 2 colsum + 1 pos + 3 resident m + 2 rotating.
    Residents eat ipairs as they land; PARKED m's burst pairs 0..3
    (resp 0..4) mid-stream into rotating banks with DVE exports, then
    finish post-stream with a short burst + add-export. The rest run
    post-stream bursts, pairs 0..4 first so only pair 5 tails.
Host finishes: colsum normalization + the small [N,262] bilinear GEMMs.
"""

import numpy as np

import concourse.tile as tile
from concourse import bacc, bass_utils, mybir

B, N, C = 4, 3072, 256
H, W = 48, 64
CP = C + 6          # 262
CX = 2 * C          # 512: [x1 256 | x2 256]
NH = N // 2         # 1536 rows per core
NT = NH // 128      # 12 row tiles per core
NP = NT // 2        # 6 DoubleRow ipairs
MT = N // 128       # 24 column tiles
CS_CHUNK = 512
NCS = N // CS_CHUNK  # 6 colsum psum chunks
CVP = CX + 16        # 528: v row: [x1 256 | x2 256 | pos 6 | pad 10]
KSC = 4096.0         # host pre-scale on v; divided back out after

RES = 3              # psum-resident m-tiles (m 0..2)
P1 = list(range(3, 7))    # parked pairs 0..3 mid-stream
P2 = list(range(7, 11))   # parked pairs 0..4 mid-stream
TAIL_A = list(range(11, 13))  # tail m's whose pairs 0..4 pre-run (2
                              # rotating banks; residents hold the rest)

FP32 = mybir.dt.float32
FP16 = mybir.dt.float16
FP8 = mybir.dt.float8e4
DR = mybir.MatmulPerfMode.DoubleRow
MUL = mybir.AluOpType.mult

TRACE = False
LAST_RESULT = None
_CACHED_NC = None


def _build_kernel():
    nc = bacc.Bacc("TRN2", target_bir_lowering=False, debug=False)
    a_in = nc.dram_tensor("a_half", [NH, N], FP8, kind="ExternalInput").ap()
    v_in = nc.dram_tensor("v_half", [128, NT * CVP], FP16, kind="ExternalInput").ap()
    b_in = nc.dram_tensor("bias_in", [128, 1], FP32, kind="ExternalInput").ap()
    x_out = nc.dram_tensor("x_out", [128, MT * CX], FP16, kind="ExternalOutput").ap()
    pos_out = nc.dram_tensor("pos_out", [128, MT * 16], FP32, kind="ExternalOutput").ap()
    cs_out = nc.dram_tensor("cs_out", [8, CS_CHUNK], FP32, kind="ExternalOutput").ap()

    with tile.TileContext(nc) as tc:
        _kernel_body(tc, a_in, v_in, b_in, x_out, pos_out, cs_out)
    nc.compile()
    return nc


def _kernel_body(tc, a_in, v_in, b_in, x_out, pos_out, cs_out):
    nc = tc.nc
    with (
        tc.tile_pool(name="singles", bufs=1) as singles,
        tc.tile_pool(name="a_pool", bufs=4) as a_pool,
        tc.tile_pool(name="e_pool", bufs=5) as e_pool,
        tc.tile_pool(name="e16_pool", bufs=2) as e16_pool,
        tc.tile_pool(name="cs_psum", bufs=1, space="PSUM") as cs_psum,
        tc.tile_pool(name="pos_psum", bufs=1, space="PSUM") as pos_psum,
        tc.tile_pool(name="x_psum", bufs=RES + 2, space="PSUM") as x_psum,
    ):
        ones_t = singles.tile([128, 1], FP16)
        nc.vector.memset(ones_t, 1.0)
        bias_t = singles.tile([128, 1], FP32)
        nc.sync.dma_start(out=bias_t, in_=b_in)

        # prefetch the exp table-set off the critical path
        dummy_t = singles.tile([128, 1], FP32)
        nc.scalar.activation(
            out=dummy_t, in_=bias_t, func=mybir.ActivationFunctionType.Exp
        )

        vk_all = singles.tile([128, NT, CVP], FP16)
        vt_all = singles.tile([128, NT, CVP], FP8)
        e2_all = singles.tile([128, NT, N], FP8)
        rowsum_all = singles.tile([128, NT + 4], FP32)
        rinv_all = singles.tile([128, NT], FP32)
        x_sb = singles.tile([128, MT, CX], FP16)
        pos_sb = singles.tile([128, MT * 16], FP32)
        cs_sb = singles.tile([128, 2, CS_CHUNK], FP32)

        cs_bank = [
            cs_psum.tile([128, CS_CHUNK], FP32, name=f"csb{t}", tag=f"csb{t}")
            for t in range(2)
        ]
        pos_bank = pos_psum.tile([128, CS_CHUNK], FP32, name="posb", tag="posb")
        for t in range(2):
            nc.vector.memset(cs_bank[t], 0.0)
        nc.vector.memset(pos_bank, 0.0)

        def cs_ap(j):
            t, p = divmod(j, 4)
            return cs_bank[t][32 * p : 32 * p + 1, :]

        def stream_chunk(i, e_t, col_lo, col_hi, accum_col):
            a_t = a_pool.tile([128, N], FP8, name="a_t", tag="a_t")
            nc.sync.dma_start(
                out=a_t[:, col_lo:col_hi],
                in_=a_in[i * 128 : (i + 1) * 128, col_lo:col_hi],
            )
            nc.scalar.activation(
                out=e_t[:, col_lo:col_hi],
                in_=a_t[:, col_lo:col_hi],
                func=mybir.ActivationFunctionType.Exp,
                bias=bias_t,
                scale=1.0,
                accum_out=rowsum_all[:, accum_col : accum_col + 1],
            )
            for j in range(col_lo // CS_CHUNK, col_hi // CS_CHUNK):
                nc.tensor.matmul(
                    cs_ap(j),
                    lhsT=ones_t,
                    rhs=e_t[:, j * CS_CHUNK : (j + 1) * CS_CHUNK],
                    start=False,
                    stop=(i == NT - 1),
                    skip_group_check=True,
                    tile_position=(0, 32 * (j % 4)),
                )

        def finish_tile(i, e_t, col_lo=0, col_hi=N):
            """square columns [col_lo:col_hi) of tile i into e2 fp8."""
            if i % 2 == 0:
                # first of the pair: TT fp16 at 2x + casting DMA -> fp8
                e16 = e16_pool.tile([128, N], FP16, name="e16", tag="e16")
                nc.vector.tensor_mul(
                    e16[:, col_lo:col_hi],
                    e_t[:, col_lo:col_hi], e_t[:, col_lo:col_hi],
                )
                nc.gpsimd.dma_start(
                    out=e2_all[:, i, col_lo:col_hi], in_=e16[:, col_lo:col_hi]
                )
            else:
                # second of the pair: straight to fp8 (1x) -- no cast
                # latency on the pair's critical path
                nc.vector.tensor_mul(
                    e2_all[:, i, col_lo:col_hi],
                    e_t[:, col_lo:col_hi], e_t[:, col_lo:col_hi],
                )

        def vt_tile(i):
            nc.vector.reciprocal(rinv_all[:, i : i + 1], rowsum_all[:, i : i + 1])
            nc.gpsimd.tensor_scalar_mul(
                vt_all[:, i, :], vk_all[:, i, :], rinv_all[:, i : i + 1]
            )

        def gemm_trio(m, p, xp, first, last, pos_last):
            lhsT = e2_all[:, 2 * p : 2 * p + 2, m * 128 : (m + 1) * 128]
            nc.tensor.matmul(
                xp[:, 0:256],
                lhsT=lhsT,
                rhs=vt_all[:, 2 * p : 2 * p + 2, 0:256],
                start=first, stop=last, perf_mode=DR, skip_group_check=True,
            )
            nc.tensor.matmul(
                xp[:, 256:512],
                lhsT=lhsT,
                rhs=vt_all[:, 2 * p : 2 * p + 2, 256:512],
                start=False, stop=last, perf_mode=DR, skip_group_check=True,
            )
            nc.tensor.matmul(
                pos_bank[:, 16 * m : 16 * (m + 1)],
                lhsT=lhsT,
                rhs=vt_all[:, 2 * p : 2 * p + 2, CX : CX + 16],
                start=False, stop=pos_last, perf_mode=DR, skip_group_check=True,
            )

        # ---- streaming phase ----------------------------------------
        res_xp = [
            x_psum.tile([128, CX], FP32, name="xp", tag="xp")
            for m in range(RES)
        ]
        for i in range(NT):
            e_t = e_pool.tile([128, N], FP16, name="e_t", tag="e_t")
            if i == 0:
                stream_chunk(0, e_t, 0, N // 2, 0)
                stream_chunk(0, e_t, N // 2, N, NT)
                nc.vector.tensor_add(
                    rowsum_all[:, 0:1], rowsum_all[:, 0:1],
                    rowsum_all[:, NT : NT + 1],
                )
                # vK load in halves, off the exp critical path
                nc.sync.dma_start(
                    out=vk_all[:, 0 : NT // 2, :],
                    in_=v_in[:, 0 : NT // 2 * CVP],
                )
            elif i == NT - 1:
                # last tile in halves so its square pipelines with the exp
                stream_chunk(i, e_t, 0, N // 2, i)
                finish_tile(i, e_t, 0, N // 2)
                stream_chunk(i, e_t, N // 2, N, NT)
                nc.vector.tensor_add(
                    rowsum_all[:, i : i + 1], rowsum_all[:, i : i + 1],
                    rowsum_all[:, NT : NT + 1],
                )
                finish_tile(i, e_t, N // 2, N)
                vt_tile(i)
            else:
                stream_chunk(i, e_t, 0, N, i)
            if i == 2:
                nc.sync.dma_start(
                    out=vk_all[:, NT // 2 : NT, :],
                    in_=v_in[:, NT // 2 * CVP : NT * CVP],
                )
            if i != NT - 1:
                finish_tile(i, e_t)
                vt_tile(i)
            if i % 2 == 1:
                p = i // 2
                for m in range(RES):
                    gemm_trio(m, p, res_xp[m], first=(p == 0),
                              last=(p == NP - 1), pos_last=(p == NP - 1))
            if i == 7:
                for m in P1:
                    xp = x_psum.tile([128, CX], FP32, name="xp", tag="xp")
                    for p in range(4):
                        gemm_trio(m, p, xp, first=(p == 0), last=(p == 3),
                                  pos_last=False)
                    nc.vector.tensor_copy(out=x_sb[:, m, :], in_=xp)
            if i == 9:
                for m in P2:
                    xp = x_psum.tile([128, CX], FP32, name="xp", tag="xp")
                    for p in range(5):
                        gemm_trio(m, p, xp, first=(p == 0), last=(p == 4),
                                  pos_last=False)
                    nc.vector.tensor_copy(out=x_sb[:, m, :], in_=xp)

        # ---- tail ------------------------------------------------------
        # tail group A: pairs 0..4 can run before pair 5's e2 lands
        tail_a_xp = {}
        for m in TAIL_A:
            xp = x_psum.tile([128, CX], FP32, name="xp", tag="xp")
            tail_a_xp[m] = xp
            for p in range(5):
                gemm_trio(m, p, xp, first=(p == 0), last=False, pos_last=False)

        for t in range(2):
            nc.scalar.copy(out=cs_sb[:, t, :], in_=cs_bank[t])
            nc.sync.dma_start(
                out=cs_out[4 * t : 4 * t + 4, :], in_=cs_sb[0:128:32, t, :]
            )

        stored = 0

        def store_upto(hi):
            nonlocal stored
            while stored + 2 <= hi:
                g = stored // 2
                nc.sync.dma_start(
                    out=x_out[:, 2 * g * CX : 2 * (g + 1) * CX],
                    in_=x_sb[:, 2 * g : 2 * (g + 1), :],
                )
                stored += 2

        for m in range(RES):
            nc.scalar.copy(out=x_sb[:, m, :], in_=res_xp[m])
        # parked finishers
        for m in P1:
            xp = x_psum.tile([128, CX], FP32, name="xp", tag="xp")
            for p in (4, 5):
                gemm_trio(m, p, xp, first=(p == 4), last=(p == 5),
                          pos_last=(p == NP - 1))
            nc.vector.tensor_add(x_sb[:, m, :], xp, x_sb[:, m, :])
        for m in P2:
            xp = x_psum.tile([128, CX], FP32, name="xp", tag="xp")
            gemm_trio(m, 5, xp, first=True, last=True, pos_last=True)
            nc.vector.tensor_add(x_sb[:, m, :], xp, x_sb[:, m, :])
        for m in TAIL_A:
            xp = tail_a_xp[m]
            gemm_trio(m, 5, xp, first=False, last=True, pos_last=(True))
            nc.scalar.copy(out=x_sb[:, m, :], in_=xp)
        store_upto(TAIL_A[-1] + 1)

        for k, m in enumerate(range(TAIL_A[-1] + 1, MT)):
            xp = x_psum.tile([128, CX], FP32, name="xp", tag="xp")
            for p in range(NP):
                gemm_trio(m, p, xp, first=(p == 0), last=(p == NP - 1),
                          pos_last=(p == NP - 1))
            if m % 2 == 0:
                nc.scalar.copy(out=x_sb[:, m, :], in_=xp)
            else:
                nc.vector.tensor_copy(out=x_sb[:, m, :], in_=xp)
            store_upto(m)
        store_upto(MT)

        nc.scalar.copy(out=pos_sb, in_=pos_bank[:, 0 : MT * 16])
        nc.sync.dma_start(out=pos_out, in_=pos_sb)


def _positional_encodings():
    ys = np.linspace(-1.0, 1.0, H, dtype=np.float32)
    xs = np.linspace(-1.0, 1.0, W, dtype=np.float32)
    p3 = np.tile(ys, W)
    p4 = np.repeat(xs, H)
    pos = np.stack([p3 * p3, p4 * p4, p3 * p4, p3, p4, np.ones_like(p3)], axis=-1)
    return pos.astype(np.float32)  # [N, 6]


def kernel(x1, x2, corr, W_proj, b_proj):
    global _CACHED_NC, LAST_RESULT
    x1 = np.asarray(x1, dtype=np.float32)
    x2 = np.asarray(x2, dtype=np.float32)
    corr = np.asarray(corr, dtype=np.float32)
    W_proj = np.asarray(W_proj, dtype=np.float32)
    b_proj = np.asarray(b_proj, dtype=np.float32)

    import ml_dtypes

    pos = _positional_encodings()
    a = corr.reshape(B, N, N).astype(ml_dtypes.float8_e4m3)
    # vK = K * [x1 | x2 | pos | pad] in fp16; the device multiplies by
    # rinv_row and quantizes to fp8
    v_all = np.zeros((B, N, CVP), dtype=np.float32)
    v_all[:, :, 0:C] = x1
    v_all[:, :, C : 2 * C] = x2
    v_all[:, :, CX : CX + 6] = np.broadcast_to(pos, (B, N, 6))
    vk = (v_all * KSC).astype(np.float16)

    if _CACHED_NC is None:
        _CACHED_NC = _build_kernel()
    nc = _CACHED_NC

    in_maps = []
    bshift = np.empty((B, 2), dtype=np.float32)
    for b in range(B):
        for h in range(2):
            rows = slice(h * NH, (h + 1) * NH)
            amax = float(a[b, rows, :].astype(np.float32).max())
            bs = 2.70 - amax       # exp(2(amax + B')) <= 221 < fp8 max 240
            bshift[b, h] = bs
            vp = (
                vk[b, rows, :]
                .reshape(NT, 128, CVP)
                .transpose(1, 0, 2)
                .reshape(128, NT * CVP)
            )
            in_maps.append(
                {
                    "a_half": np.ascontiguousarray(a[b, rows, :]),
                    "v_half": np.ascontiguousarray(vp),
                    "bias_in": np.full((128, 1), bs, dtype=np.float32),
                }
            )

    res = bass_utils.run_bass_kernel_spmd(
        nc, in_maps, core_ids=list(range(8)), trace=TRACE
    )
    LAST_RESULT = res

    v1 = np.concatenate([x1, np.broadcast_to(pos, (B, N, 6))], axis=2)
    v2 = np.concatenate([x2, np.broadcast_to(pos, (B, N, 6))], axis=2)

    out1 = np.empty((B, CP, C), dtype=np.float32)
    out2 = np.empty((B, CP, C), dtype=np.float32)
    for b in range(B):
        r0, r1 = res.results[2 * b], res.results[2 * b + 1]
        # per-half compensation: X_true_h = X_h * e^{-B'_h} / K
        s0 = np.exp(-bshift[b, 0]) / KSC
        s1 = np.exp(-bshift[b, 1]) / KSC
        X = (
            r0["x_out"].astype(np.float32) * s0
            + r1["x_out"].astype(np.float32) * s1
        ).reshape(128, MT, CX).transpose(1, 0, 2).reshape(N, CX)
        pos_raw = (
            r0["pos_out"] * s0 + r1["pos_out"] * s1
        )   # [128, MT*16]
        pos_x = (
            pos_raw.reshape(128, MT, 16)[:, :, 0:6]
            .transpose(1, 0, 2)
            .reshape(N, 6)
        )
        # colsum: C0 = sum_h cs_h * e^{-B'_h}
        cs0 = np.exp(-bshift[b, 0])
        cs1 = np.exp(-bshift[b, 1])
        colsum = np.empty(N, dtype=np.float32)
        for j in range(NCS):
            t, p = divmod(j, 4)
            colsum[j * CS_CHUNK : (j + 1) * CS_CHUNK] = (
                r0["cs_out"][4 * t + p] * cs0 + r1["cs_out"][4 * t + p] * cs1
            )
        c = 1.0 / colsum
        vc1 = v1[b] * c[:, None]
        vc2 = v2[b] * c[:, None]
        X1 = np.concatenate([X[:, 0:256], pos_x], axis=1)   # [N, 262]
        X2 = np.concatenate([X[:, 256:512], pos_x], axis=1)
        fund1 = X1.T @ vc1      # [262, 262] = v1^T attn v1
        fund2t = X2.T @ vc2     # = (v2^T attn^T v2)^T
        out1[b] = fund1.T @ W_proj + b_proj
        out2[b] = fund2t @ W_proj + b_proj
    return (out2, out1)


# revision 25
# speedup vs baseline: 1.8071x; 1.8071x over previous
"""Trainium2 Bass kernel: dual-softmax cross-attention bilinear forms.

Math (per batch b, a = corr[b] in [N, N], N = 3072):
    attn = exp(2a) * (1/rowsum_a) outer (1/colsum_a)
    fund1 = v1^T attn v1,  fund2^T = v2^T attn^T v2
Device computes, per core (4 batches x 2 row-halves = 8 cores), for its
half slab [NH=1536, N]:
    E1 = exp(a + B') fp16 on the scalar engine; B' = 2.70 - amax is set
    per core on the host so E1^2 <= 221 < 240 (fp8e4 max -- the type
    has an inf, it is NOT e4m3fn). a arrives fp8 (noise washes out in
    the dual normalization). rowsum via the activation accumulator;
    colsum partials via ones^T @ E1 matmuls on 4 PE column groups.
    e2, per ipair (2 tiles), mixed so the DVE stays under the stream
    budget (measured: STT->fp8 3.41us, TT->fp16 1.71us at 2x_1P,
    TT->fp8 ~8us (avoid), gpsimd tensor_scalar ~8.6us (avoid)):
      even tile: e2 = fp16 TT square -> GPSIMD casting DMA -> fp8
        (bit-exact RNE), row-normalization carried by the v side:
        vt_even = fp8(v * 512 * rinv) -- one small DVE tensor_scalar.
      odd tile:  e2 = fp8 STT (E1 * (16/R)) * E1 -- normalization
        inside e2; its v side is host-quantized vr8 = fp8(32 * v).
      Both contribute 512 * E1^2 * rinv * v to X; host divides by
      512 * e^{B'} per half. The cast latency never gates a pair
      (even = first tile); the last tile (odd) runs split in halves.
    X = e2^T @ vt -- fp8 DoubleRow matmul trios per (m, ipair):
    [0:256 | 256:512 | pos 16-wide window in a shared bank], ~278ns
    warm. PSUM (8 banks): 2 colsum + 1 pos + 3 resident m + 2 rotating.
    Residents eat ipairs as they land; parked m's burst pairs 0..3/0..4
    mid-stream with DVE exports and finish post-stream with short
    bursts + add-exports; two more pre-run pairs 0..4 right after the
    stream; the rest run dense full bursts on the hot PE.
Host finishes: colsum normalization + the small [N,262] bilinear GEMMs.
"""

import numpy as np

import concourse.tile as tile
from concourse import bacc, bass_utils, mybir

B, N, C = 4, 3072, 256
H, W = 48, 64
CP = C + 6          # 262
CX = 2 * C          # 512: [x1 256 | x2 256]
NH = N // 2         # 1536 rows per core
NT = NH // 128      # 12 row tiles per core
NP = NT // 2        # 6 DoubleRow ipairs
MT = N // 128       # 24 column tiles
CS_CHUNK = 512
NCS = N // CS_CHUNK  # 6 colsum psum chunks
CVP = CX + 16        # 528: v row: [x1 256 | x2 256 | pos 6 | pad 10]
KV16 = 512.0         # host pre-scale on the fp16 v (even-tile path)
KV8 = 2.0            # host pre-scale on the fp8 v (odd-tile path)
KR = 256.0           # odd-tile STT scalar = KR / rowsum; KR*KV8 = KV16.
                     # KR centers e2r in the fp8 window: measured raw max
                     # ~110 < 240 on randn slabs (rows never concentrate)

RES = 3                   # psum-resident m-tiles (m 0..2)
P1 = [3, 4]               # parked pairs 0..3 mid-stream
P2 = [5, 6]               # parked pairs 0..4 mid-stream
TAIL_A = [7, 8]           # pairs 0..4 pre-run right after the stream

FP32 = mybir.dt.float32
FP16 = mybir.dt.float16
FP8 = mybir.dt.float8e4
DR = mybir.MatmulPerfMode.DoubleRow
MUL = mybir.AluOpType.mult

TRACE = False
LAST_RESULT = None
_CACHED_NC = None


def _build_kernel():
    nc = bacc.Bacc("TRN2", target_bir_lowering=False, debug=False)
    a_in = nc.dram_tensor("a_half", [NH, N], FP8, kind="ExternalInput").ap()
    v_in = nc.dram_tensor("v_half", [128, NP * CVP], FP16, kind="ExternalInput").ap()
    v8_in = nc.dram_tensor("v8_half", [128, NP * CVP], FP8, kind="ExternalInput").ap()
    b_in = nc.dram_tensor("bias_in", [128, 1], FP32, kind="ExternalInput").ap()
    x_out = nc.dram_tensor("x_out", [128, MT * CX], FP16, kind="ExternalOutput").ap()
    pos_out = nc.dram_tensor("pos_out", [128, MT * 16], FP32, kind="ExternalOutput").ap()
    cs_out = nc.dram_tensor("cs_out", [8, CS_CHUNK], FP32, kind="ExternalOutput").ap()

    with tile.TileContext(nc) as tc:
        _kernel_body(tc, a_in, v_in, v8_in, b_in, x_out, pos_out, cs_out)
    nc.compile()
    return nc


def _kernel_body(tc, a_in, v_in, v8_in, b_in, x_out, pos_out, cs_out):
    nc = tc.nc
    with (
        tc.tile_pool(name="singles", bufs=1) as singles,
        tc.tile_pool(name="a_pool", bufs=4) as a_pool,
        tc.tile_pool(name="e_pool", bufs=5) as e_pool,
        tc.tile_pool(name="e16_pool", bufs=2) as e16_pool,
        tc.tile_pool(name="cs_psum", bufs=1, space="PSUM") as cs_psum,
        tc.tile_pool(name="pos_psum", bufs=1, space="PSUM") as pos_psum,
        tc.tile_pool(name="x_psum", bufs=RES + 2, space="PSUM") as x_psum,
    ):
        ones_t = singles.tile([128, 1], FP16)
        nc.vector.memset(ones_t, 1.0)
        bias_t = singles.tile([128, 1], FP32)
        nc.sync.dma_start(out=bias_t, in_=b_in)

        # prefetch the exp table-set off the critical path
        dummy_t = singles.tile([128, 1], FP32)
        nc.scalar.activation(
            out=dummy_t, in_=bias_t, func=mybir.ActivationFunctionType.Exp
        )

        vk_all = singles.tile([128, NP, CVP], FP16)   # even tiles' v*512
        vt_all = singles.tile([128, NT, CVP], FP8)
        e2_all = singles.tile([128, NT, N], FP8)
        rowsum_all = singles.tile([128, NT + 4], FP32)
        rinv_all = singles.tile([128, NT], FP32)
        x_sb = singles.tile([128, MT, CX], FP16)
        pos_sb = singles.tile([128, MT * 16], FP32)
        cs_sb = singles.tile([128, 2, CS_CHUNK], FP32)

        cs_bank = [
            cs_psum.tile([128, CS_CHUNK], FP32, name=f"csb{t}", tag=f"csb{t}")
            for t in range(2)
        ]
        pos_bank = pos_psum.tile([128, CS_CHUNK], FP32, name="posb", tag="posb")
        for t in range(2):
            nc.vector.memset(cs_bank[t], 0.0)
        nc.vector.memset(pos_bank, 0.0)

        def cs_ap(j):
            t, p = divmod(j, 4)
            return cs_bank[t][32 * p : 32 * p + 1, :]

        def stream_chunk(i, e_t, col_lo, col_hi, accum_col):
            a_t = a_pool.tile([128, N], FP8, name="a_t", tag="a_t")
            nc.sync.dma_start(
                out=a_t[:, col_lo:col_hi],
                in_=a_in[i * 128 : (i + 1) * 128, col_lo:col_hi],
            )
            nc.scalar.activation(
                out=e_t[:, col_lo:col_hi],
                in_=a_t[:, col_lo:col_hi],
                func=mybir.ActivationFunctionType.Exp,
                bias=bias_t,
                scale=1.0,
                accum_out=rowsum_all[:, accum_col : accum_col + 1],
            )
            for j in range(col_lo // CS_CHUNK, col_hi // CS_CHUNK):
                nc.tensor.matmul(
                    cs_ap(j),
                    lhsT=ones_t,
                    rhs=e_t[:, j * CS_CHUNK : (j + 1) * CS_CHUNK],
                    start=False,
                    stop=(i == NT - 1),
                    skip_group_check=True,
                    tile_position=(0, 32 * (j % 4)),
                )

        def finish_even(i, e_t):
            """even tile: recip; vt = vK*rinv; e2 via fp16 TT + cast."""
            nc.vector.reciprocal(rinv_all[:, i : i + 1], rowsum_all[:, i : i + 1])
            nc.vector.tensor_scalar_mul(
                vt_all[:, i, :], vk_all[:, i // 2, :], rinv_all[:, i : i + 1]
            )
            e16 = e16_pool.tile([128, N], FP16, name="e16", tag="e16")
            nc.vector.tensor_mul(e16, e_t, e_t)
            nc.gpsimd.dma_start(out=e2_all[:, i, :], in_=e16)

        def finish_odd(i, e_t, col_lo=0, col_hi=N):
            """odd tile: e2 = (E1*(16/R))*E1 straight to fp8 (STT, 1x)."""
            nc.vector.scalar_tensor_tensor(
                out=e2_all[:, i, col_lo:col_hi],
                in0=e_t[:, col_lo:col_hi],
                scalar=rinv_all[:, i : i + 1],
                in1=e_t[:, col_lo:col_hi],
                op0=MUL,
                op1=MUL,
            )

        def odd_rinv(i):
            nc.vector.reciprocal(rinv_all[:, i : i + 1], rowsum_all[:, i : i + 1])
            nc.vector.tensor_scalar_mul(
                rinv_all[:, i : i + 1], rinv_all[:, i : i + 1], KR
            )

        def gemm_trio(m, p, xp, first, last, pos_last):
            lhsT = e2_all[:, 2 * p : 2 * p + 2, m * 128 : (m + 1) * 128]
            nc.tensor.matmul(
                xp[:, 0:256],
                lhsT=lhsT,
                rhs=vt_all[:, 2 * p : 2 * p + 2, 0:256],
                start=first, stop=last, perf_mode=DR, skip_group_check=True,
            )
            nc.tensor.matmul(
                xp[:, 256:512],
                lhsT=lhsT,
                rhs=vt_all[:, 2 * p : 2 * p + 2, 256:512],
                start=False, stop=last, perf_mode=DR, skip_group_check=True,
            )
            nc.tensor.matmul(
                pos_bank[:, 16 * m : 16 * (m + 1)],
                lhsT=lhsT,
                rhs=vt_all[:, 2 * p : 2 * p + 2, CX : CX + 16],
                start=False, stop=pos_last, perf_mode=DR, skip_group_check=True,
            )

        # ---- streaming phase ----------------------------------------
        res_xp = [
            x_psum.tile([128, CX], FP32, name="xp", tag="xp")
            for m in range(RES)
        ]
        for i in range(NT):
            e_t = e_pool.tile([128, N], FP16, name="e_t", tag="e_t")
            if i == 0:
                stream_chunk(0, e_t, 0, N // 2, 0)
                stream_chunk(0, e_t, N // 2, N, NT)
                nc.vector.tensor_add(
                    rowsum_all[:, 0:1], rowsum_all[:, 0:1],
                    rowsum_all[:, NT : NT + 1],
                )
                # v loads ride after tile 0, off the exp critical path
                nc.sync.dma_start(out=vk_all, in_=v_in)
                # odd tiles' fp8 v goes straight into vt slots 1,3,..
                nc.sync.dma_start(
                    out=vt_all[:, 1 : NT : 2, :], in_=v8_in
                )
            elif i == NT - 1:
                # last tile (odd) in halves: its STT pipelines with exp
                stream_chunk(i, e_t, 0, N // 2, i)
                stream_chunk(i, e_t, N // 2, N, NT)
            else:
                stream_chunk(i, e_t, 0, N, i)
            if i == NT - 1:
                nc.vector.tensor_add(
                    rowsum_all[:, i : i + 1], rowsum_all[:, i : i + 1],
                    rowsum_all[:, NT : NT + 1],
                )
                odd_rinv(i)
                finish_odd(i, e_t, 0, N // 2)
                finish_odd(i, e_t, N // 2, N)
            elif i % 2 == 0:
                finish_even(i, e_t)
            else:
                odd_rinv(i)
                finish_odd(i, e_t)
            if i % 2 == 1:
                p = i // 2
                for m in range(RES):
                    gemm_trio(m, p, res_xp[m], first=(p == 0),
                              last=(p == NP - 1), pos_last=(p == NP - 1))
            if i == 7:
                for m in P1:
                    xp = x_psum.tile([128, CX], FP32, name="xp", tag="xp")
                    for p in range(4):
                        gemm_trio(m, p, xp, first=(p == 0), last=(p == 3),
                                  pos_last=False)
                    nc.vector.tensor_copy(out=x_sb[:, m, :], in_=xp)
            if i == 9:
                for m in P2:
                    xp = x_psum.tile([128, CX], FP32, name="xp", tag="xp")
                    for p in range(5):
                        gemm_trio(m, p, xp, first=(p == 0), last=(p == 4),
                                  pos_last=False)
                    nc.vector.tensor_copy(out=x_sb[:, m, :], in_=xp)

        # ---- tail ------------------------------------------------------
        # pre-run pairs 0..4 for TAIL_A while pair 5 finishes
        tail_a_xp = {}
        for m in TAIL_A:
            xp = x_psum.tile([128, CX], FP32, name="xp", tag="xp")
            tail_a_xp[m] = xp
            for p in range(5):
                gemm_trio(m, p, xp, first=(p == 0), last=False, pos_last=False)

        for t in range(2):
            nc.scalar.copy(out=cs_sb[:, t, :], in_=cs_bank[t])
            nc.sync.dma_start(
                out=cs_out[4 * t : 4 * t + 4, :], in_=cs_sb[0:128:32, t, :]
            )

        stored = 0

        def store_upto(hi):
            nonlocal stored
            while stored + 2 <= hi:
                g = stored // 2
                nc.sync.dma_start(
                    out=x_out[:, 2 * g * CX : 2 * (g + 1) * CX],
                    in_=x_sb[:, 2 * g : 2 * (g + 1), :],
                )
                stored += 2

        for m in range(RES):
            nc.scalar.copy(out=x_sb[:, m, :], in_=res_xp[m])
        for m in P1:
            xp = x_psum.tile([128, CX], FP32, name="xp", tag="xp")
            for p in (4, 5):
                gemm_trio(m, p, xp, first=(p == 4), last=(p == 5),
                          pos_last=(p == NP - 1))
            nc.vector.tensor_add(x_sb[:, m, :], xp, x_sb[:, m, :])
        for m in P2:
            xp = x_psum.tile([128, CX], FP32, name="xp", tag="xp")
            gemm_trio(m, 5, xp, first=True, last=True, pos_last=True)
            nc.vector.tensor_add(x_sb[:, m, :], xp, x_sb[:, m, :])
        for m in TAIL_A:
            xp = tail_a_xp[m]
            gemm_trio(m, 5, xp, first=False, last=True, pos_last=True)
            nc.scalar.copy(out=x_sb[:, m, :], in_=xp)
        store_upto(TAIL_A[-1] + 1)

        for m in range(TAIL_A[-1] + 1, MT):
            xp = x_psum.tile([128, CX], FP32, name="xp", tag="xp")
            for p in range(NP):
                gemm_trio(m, p, xp, first=(p == 0), last=(p == NP - 1),
                          pos_last=(p == NP - 1))
            if m % 2 == 0:
                nc.scalar.copy(out=x_sb[:, m, :], in_=xp)
            else:
                nc.vector.tensor_copy(out=x_sb[:, m, :], in_=xp)
            store_upto(m)
        store_upto(MT)

        nc.scalar.copy(out=pos_sb, in_=pos_bank[:, 0 : MT * 16])
        nc.sync.dma_start(out=pos_out, in_=pos_sb)


def _positional_encodings():
    ys = np.linspace(-1.0, 1.0, H, dtype=np.float32)
    xs = np.linspace(-1.0, 1.0, W, dtype=np.float32)
    p3 = np.tile(ys, W)
    p4 = np.repeat(xs, H)
    pos = np.stack([p3 * p3, p4 * p4, p3 * p4, p3, p4, np.ones_like(p3)], axis=-1)
    return pos.astype(np.float32)  # [N, 6]


def kernel(x1, x2, corr, W_proj, b_proj):
    global _CACHED_NC, LAST_RESULT
    x1 = np.asarray(x1, dtype=np.float32)
    x2 = np.asarray(x2, dtype=np.float32)
    corr = np.asarray(corr, dtype=np.float32)
    W_proj = np.asarray(W_proj, dtype=np.float32)
    b_proj = np.asarray(b_proj, dtype=np.float32)

    import ml_dtypes

    pos = _positional_encodings()
    f8 = ml_dtypes.float8_e4m3
    a = corr.reshape(B, N, N).astype(f8)
    v_all = np.zeros((B, N, CVP), dtype=np.float32)
    v_all[:, :, 0:C] = x1
    v_all[:, :, C : 2 * C] = x2
    v_all[:, :, CX : CX + 6] = np.broadcast_to(pos, (B, N, 6))
    vk = (v_all * KV16).astype(np.float16)
    v8 = (v_all * KV8).astype(f8)

    if _CACHED_NC is None:
        _CACHED_NC = _build_kernel()
    nc = _CACHED_NC

    in_maps = []
    bshift = np.empty((B, 2), dtype=np.float32)
    for b in range(B):
        for h in range(2):
            rows = slice(h * NH, (h + 1) * NH)
            amax = float(a[b, rows, :].astype(np.float32).max())
            bs = 2.70 - amax       # exp(2(amax + B')) <= 221 < fp8 max 240
            bshift[b, h] = bs
            # even tiles' fp16 v, packed partition-major per tile pair:
            # vk slot k holds tile 2k
            vkh = vk[b, rows, :].reshape(NT, 128, CVP)
            vk_even = (
                vkh[0::2].transpose(1, 0, 2).reshape(128, NP * CVP)
            )
            v8h = v8[b, rows, :].reshape(NT, 128, CVP)
            v8_odd = (
                v8h[1::2].transpose(1, 0, 2).reshape(128, NP * CVP)
            )
            in_maps.append(
                {
                    "a_half": np.ascontiguousarray(a[b, rows, :]),
                    "v_half": np.ascontiguousarray(vk_even),
                    "v8_half": np.ascontiguousarray(v8_odd),
                    "bias_in": np.full((128, 1), bs, dtype=np.float32),
                }
            )

    res = bass_utils.run_bass_kernel_spmd(
        nc, in_maps, core_ids=list(range(8)), trace=TRACE
    )
    LAST_RESULT = res

    v1 = np.concatenate([x1, np.broadcast_to(pos, (B, N, 6))], axis=2)
    v2 = np.concatenate([x2, np.broadcast_to(pos, (B, N, 6))], axis=2)

    out1 = np.empty((B, CP, C), dtype=np.float32)
    out2 = np.empty((B, CP, C), dtype=np.float32)
    for b in range(B):
        r0, r1 = res.results[2 * b], res.results[2 * b + 1]
        # per-half compensation: X_true_h = X_h * e^{-B'_h} / 512
        s0 = np.exp(-bshift[b, 0]) / KV16
        s1 = np.exp(-bshift[b, 1]) / KV16
        X = (
            r0["x_out"].astype(np.float32) * s0
            + r1["x_out"].astype(np.float32) * s1
        ).reshape(128, MT, CX).transpose(1, 0, 2).reshape(N, CX)
        pos_raw = r0["pos_out"] * s0 + r1["pos_out"] * s1
        pos_x = (
            pos_raw.reshape(128, MT, 16)[:, :, 0:6]
            .transpose(1, 0, 2)
            .reshape(N, 6)
        )
        cs0 = np.exp(-bshift[b, 0])
        cs1 = np.exp(-bshift[b, 1])
        colsum = np.empty(N, dtype=np.float32)
        for j in range(NCS):
            t, p = divmod(j, 4)
            colsum[j * CS_CHUNK : (j + 1) * CS_CHUNK] = (
                r0["cs_out"][4 * t + p] * cs0 + r1["cs_out"][4 * t + p] * cs1
            )
        c = 1.0 / colsum
        vc1 = v1[b] * c[:, None]
        vc2 = v2[b] * c[:, None]
        X1 = np.concatenate([X[:, 0:256], pos_x], axis=1)   # [N, 262]
        X2 = np.concatenate([X[:, 256:512], pos_x], axis=1)
        fund1 = X1.T @ vc1      # [262, 262] = v1^T attn v1
        fund2t = X2.T @ vc2     # = (v2^T attn^T v2)^T
        out1[b] = fund1.T @ W_proj + b_proj
        out2[b] = fund2t @ W_proj + b_proj
    return (out2, out1)
